# revision 6
# baseline (speedup 1.0000x reference)
"""Child-Sum TreeLSTM over a complete 8-ary tree (depth 6, 299593 nodes) on
8 Trainium2 NeuronCores.

Sharding: the 8 independent subtrees under the root go one-per-core; each core
runs the bottom-up sweep for levels 6 (leaves), 5 and 4 of its subtree and
returns (c4, sigmoid(o4)) for its 512 level-4 nodes. The top of the tree
(levels 3..1 = 73 nodes/subtree plus the root step) runs on the host in fp32
after the gather -- those levels are a deeply serial dependency chain that
wastes ~30us of device wall-clock for <1% of the FLOPs.

Device layout: feature-major ([128 features on partitions, nodes on free dim]),
x pre-transposed/cast to fp16 on the host. The kernel is ScalarE(ACT)-bound:
~173k activation columns/core at 1 col/cycle @1.2GHz. Matmuls run in fp16
(fp32 PSUM), gates via ACT (sigmoid/tanh, per-partition bias APs), child-sum
reductions as fp16 tree-adds on the vector engine, merged 4096-wide where
possible. For OFFLOAD_CHUNKS leaf chunks, tanh(c_leaf) is evaluated on the
vector engine instead of ACT via an odd polynomial x*(c0*y^2+c1*y+c2), y=x^2
(valid since c_leaf = sigmoid(i)*tanh(u) is in (-1,1); max err 1.7e-3), which
trades ~1.75 DVE cyc/col for 1 ACT cyc/col to rebalance the two engines.
"""

import os

import numpy as np

import concourse.bass as bass
import concourse.tile as tile
from concourse import bacc, mybir
from concourse.bass_utils import run_bass_kernel_spmd

F16 = mybir.dt.float16
F32 = mybir.dt.float32
SIG = mybir.ActivationFunctionType.Sigmoid
TANH = mybir.ActivationFunctionType.Tanh
ADD = mybir.AluOpType.add
MULT = mybir.AluOpType.mult

BRANCH = 8
DEPTH = 6
MEM = 128
IN_DIM = 128
N_NODES = (BRANCH ** (DEPTH + 1) - 1) // (BRANCH - 1)  # 299593

LVL_SIZES = [BRANCH**i for i in range(DEPTH)]  # [1, 8, 64, 512, 4096, 32768]
LVL_OFF = [sum(LVL_SIZES[:i]) for i in range(DEPTH)]  # [0,1,9,73,585,4681]
SUB_N = sum(LVL_SIZES)  # 37449

LEAF_OFF = LVL_OFF[5]  # 4681
L5_OFF = LVL_OFF[4]  # 585
L4_OFF = LVL_OFF[3]  # 73
N_CHUNKS = 8  # leaf chunks of 4096 leaves (512 L5 parents each)

# Leaf chunks whose tanh(c) runs on the vector engine (polynomial) instead of
# the ACT engine: each moves ~3.4us off ACT and ~7.5us onto DVE.
OFFLOAD_CHUNKS = tuple(
    int(c)
    for c in os.environ.get("TREELSTM_OFFLOAD", "1,2,3,4").split(",")
    if c != ""
)

# x*(TP0*y^2 + TP1*y + TP2), y = x*x: tanh on [-1,1], max abs err 1.7e-3.
TP0 = 0.07871904
TP1 = -0.31408396
TP2 = 0.99836299

LAST_RESULTS = None  # stash for test harness introspection


def _build_subtree_kernel():
    nc = bacc.Bacc("TRN2", target_bir_lowering=False, debug=False, num_devices=8)

    xs = nc.dram_tensor("xs", [128, SUB_N], F16, kind="ExternalInput").ap()
    wioux_d = nc.dram_tensor("wioux", [128, 384], F16, kind="ExternalInput").ap()
    wiouh_d = nc.dram_tensor("wiouh", [128, 384], F16, kind="ExternalInput").ap()
    wfx_d = nc.dram_tensor("wfx", [128, 128], F16, kind="ExternalInput").ap()
    wfh_d = nc.dram_tensor("wfh", [128, 128], F16, kind="ExternalInput").ap()
    biou_d = nc.dram_tensor("biou", [128, 3], F32, kind="ExternalInput").ap()
    biourow_d = nc.dram_tensor("biourow", [1, 384], F16, kind="ExternalInput").ap()
    bf_d = nc.dram_tensor("bf", [128, 1], F32, kind="ExternalInput").ap()
    # cols 0:512 = c4 (fp32), cols 512:1024 = sigmoid(o4) (fp32)
    out_d = nc.dram_tensor("out", [128, 1024], F32, kind="ExternalOutput").ap()

    with tile.TileContext(nc) as tc:
        with (
            tc.tile_pool(name="const", bufs=1) as cp,
            tc.tile_pool(name="xlo", bufs=1) as xlo_p,
            tc.tile_pool(name="x6", bufs=3) as x6_p,
            tc.tile_pool(name="leafst", bufs=2) as lf_p,
            tc.tile_pool(name="state", bufs=1) as st,
            tc.tile_pool(name="gates", bufs=2) as gp,
            tc.tile_pool(name="gq", bufs=1) as gq,
            tc.tile_pool(name="psum", bufs=2, space="PSUM") as psum,
        ):
            W = {}

            def load_const(name, dram, shape, dt, engine):
                t = cp.tile(shape, dt, tag=name)
                engine.dma_start(t[:], dram)
                W[name] = t

            # x6_tiles[(ch, block_start)] = tile covering that leaf block
            x6_tiles = {}
            # first 512 leaf cols ASAP on the sync queue, then constants
            t0a = x6_p.tile([128, 512], F16, tag="x6", name="x6_0a")
            nc.sync.dma_start(t0a[:], xs[:, LEAF_OFF : LEAF_OFF + 512])
            x6_tiles[(0, 0)] = t0a
            load_const("wioux", wioux_d, [128, 384], F16, nc.sync)
            load_const("biou", biou_d, [128, 3], F32, nc.sync)
            t0b = x6_p.tile([128, 1536], F16, tag="x6", name="x6_0b")
            nc.sync.dma_start(t0b[:], xs[:, LEAF_OFF + 512 : LEAF_OFF + 2048])
            x6_tiles[(0, 512)] = t0b

            load_const("wiouh", wiouh_d, [128, 384], F16, nc.gpsimd)
            load_const("wfx", wfx_d, [128, 128], F16, nc.gpsimd)
            load_const("wfh", wfh_d, [128, 128], F16, nc.gpsimd)
            load_const("biourow", biourow_d, [1, 384], F16, nc.gpsimd)
            load_const("bf", bf_d, [128, 1], F32, nc.gpsimd)
            ones = cp.tile([1, 512], F16, tag="ones")
            nc.vector.memset(ones[:], 1.0)
            W["ones"] = ones

            # x for levels L4..L5 (+ unused L1..L3 head), persistent
            x15 = xlo_p.tile([128, LEAF_OFF], F16)
            nc.gpsimd.dma_start(x15[:], xs[:, 0:LEAF_OFF])

            # persistent state/partials
            hs5 = st.tile([128, 4096], F16, tag="hs5")
            fc5 = st.tile([128, 4096], F16, tag="fc5")
            c5 = st.tile([128, 4096], F16, tag="c5")
            h5 = st.tile([128, 4096], F16, tag="h5")
            hs4 = st.tile([128, 512], F16, tag="hs4")
            fc4 = st.tile([128, 512], F16, tag="fc4")
            outt = st.tile([128, 1024], F32, tag="outt")

            bi = W["biou"][:, 0:1]
            bo = W["biou"][:, 1:2]
            bu = W["biou"][:, 2:3]

            # ---- leaf chunks: iou gates for 4096 leaves -------------------
            leaf_states = {}

            def leaf_chunk(ch):
                lc = lf_p.tile([128, 4096], F16, tag="lc", name=f"lc{ch}")
                lh = lf_p.tile([128, 4096], F16, tag="lh", name=f"lh{ch}")
                leaf_states[ch] = (lc, lh)
                si4 = gp.tile([128, 4096], F16, tag="si4", name=f"si4_{ch}")
                tu4 = gp.tile([128, 4096], F16, tag="tu4", name=f"tu4_{ch}")
                so4 = gp.tile([128, 4096], F16, tag="so4", name=f"so4_{ch}")
                blocks = [(0, 512), (512, 1536)] if ch == 0 else [(0, 2048)]
                blocks.append((2048, 2048))
                base = LEAF_OFF + ch * 4096
                for bs, bn in blocks:
                    if (ch, bs) not in x6_tiles:
                        t = x6_p.tile([128, bn], F16, tag="x6", name=f"x6_{ch}_{bs}")
                        nc.sync.dma_start(t[:], xs[:, base + bs : base + bs + bn])
                        x6_tiles[(ch, bs)] = t
                    x6t = x6_tiles[(ch, bs)]

                    def gate_psum(gate, name):
                        p = psum.tile([128, bn], F32, tag="pg", name=name)
                        w = W["wioux"][:, gate * 128 : (gate + 1) * 128]
                        for s in range(0, bn, 512):
                            nc.tensor.matmul(
                                p[:, s : s + 512], w, x6t[:, s : s + 512],
                                start=True, stop=True,
                            )
                        return p

                    sl = slice(bs, bs + bn)
                    pi = gate_psum(0, f"pi6_{ch}_{bs}")
                    pu = gate_psum(2, f"pu6_{ch}_{bs}")
                    nc.scalar.activation(si4[:, sl], pi[:], SIG, bias=bi)
                    nc.scalar.activation(tu4[:, sl], pu[:], TANH, bias=bu)
                    po = gate_psum(1, f"po6_{ch}_{bs}")
                    nc.scalar.activation(so4[:, sl], po[:], SIG, bias=bo)

                nc.vector.tensor_mul(lc[:], si4[:], tu4[:])
                tct = gp.tile([128, 4096], F16, tag="f4", name=f"tct4_{ch}")
                if ch in OFFLOAD_CHUNKS:
                    # tanh(lc) via odd polynomial on the vector engine,
                    # half-chunk at a time (lc is in (-1,1): no clamp needed)
                    for h in range(2):
                        sl = slice(h * 2048, (h + 1) * 2048)
                        y = gq.tile([128, 2048], F16, tag="py", name=f"py_{ch}_{h}")
                        t = gq.tile([128, 2048], F16, tag="pt", name=f"pt_{ch}_{h}")
                        u = gq.tile([128, 2048], F16, tag="pu", name=f"pu_{ch}_{h}")
                        w = gq.tile([128, 2048], F16, tag="py", name=f"pw_{ch}_{h}")
                        nc.vector.tensor_mul(y[:], lc[:, sl], lc[:, sl])
                        nc.vector.tensor_scalar(t[:], y[:], TP0, TP1, MULT, ADD)
                        nc.vector.tensor_mul(u[:], t[:], y[:])
                        nc.vector.tensor_scalar_add(w[:], u[:], TP2)
                        nc.vector.tensor_mul(tct[:, sl], w[:], lc[:, sl])
                else:
                    nc.scalar.activation(tct[:], lc[:], TANH)
                nc.vector.tensor_mul(lh[:], so4[:], tct[:])

            # ---- child-sum tree adds (groups of 8) ------------------------
            def _tree(src_ap, n_par, dst, name):
                s3 = src_ap.rearrange("p (m f) -> p m f", f=8)
                t1 = gq.tile([128, 4 * n_par], F16, tag="t1", name=f"t1_{name}")
                t1v = t1[:].rearrange("p (m f) -> p m f", f=4)
                nc.vector.tensor_add(t1v, s3[:, :, 0:4], s3[:, :, 4:8])
                t2 = gq.tile([128, 2 * n_par], F16, tag="t2", name=f"t2_{name}")
                t2v = t2[:].rearrange("p (m f) -> p m f", f=2)
                nc.vector.tensor_add(t2v, t1v[:, :, 0:2], t1v[:, :, 2:4])
                dstv = dst.rearrange("p (m f) -> p m f", f=1)
                nc.vector.tensor_add(dstv, t2v[:, :, 0:1], t2v[:, :, 1:2])

            # ---- L5 forget gates + child sums for one chunk (512 parents) -
            def gates5(ch):
                lc, lh = leaf_states.pop(ch)
                f4 = gp.tile([128, 4096], F16, tag="f4", name=f"f4_{ch}")
                for h in range(2):
                    cols = slice(h * 2048, (h + 1) * 2048)
                    pf = psum.tile([128, 2048], F32, tag="pg", name=f"pf5_{ch}_{h}")
                    for s in range(0, 2048, 512):
                        nc.tensor.matmul(
                            pf[:, s : s + 512],
                            W["wfh"][:],
                            lh[:, h * 2048 + s : h * 2048 + s + 512],
                            start=True, stop=False,
                        )
                    for s in range(0, 2048, 512):
                        ps = ch * 512 + h * 256 + s // 8
                        xb = (
                            x15[:, L5_OFF + ps : L5_OFF + ps + 64]
                            .rearrange("p (m o) -> p m o", o=1)
                            .broadcast_to([128, 64, 8])
                        )
                        nc.tensor.matmul(
                            pf[:, s : s + 512], W["wfx"][:], xb,
                            start=False, stop=True,
                        )
                    nc.scalar.activation(f4[:, cols], pf[:], SIG, bias=W["bf"][:])
                prod = gq.tile([128, 4096], F16, tag="prod", name=f"prod_{ch}")
                nc.vector.tensor_mul(prod[:], f4[:], lc[:])
                psl = slice(ch * 512, (ch + 1) * 512)
                _tree(prod[:], 512, fc5[:, psl], f"fc_{ch}")
                _tree(lh[:], 512, hs5[:, psl], f"hs_{ch}")

            # ---- L5 iou gates + cell update, per quarter (1024 parents) ---
            def _iou_psum(gate, sl, name):
                n = sl.stop - sl.start
                p = psum.tile([128, n], F32, tag="pg", name=name)
                w = W["wioux"][:, gate * 128 : (gate + 1) * 128]
                x_l = x15[:, L5_OFF + sl.start : L5_OFF + sl.stop]
                for s in range(0, n, 512):
                    nc.tensor.matmul(
                        p[:, s : s + 512], w, x_l[:, s : s + 512],
                        start=True, stop=False,
                    )
                wh = W["wiouh"][:, gate * 128 : (gate + 1) * 128]
                for s in range(0, n, 512):
                    nc.tensor.matmul(
                        p[:, s : s + 512], wh, hs5[:, sl][:, s : s + 512],
                        start=False, stop=True,
                    )
                return p

            def l5top_p1(q):
                sl = slice(q * 1024, (q + 1) * 1024)
                pi = _iou_psum(0, sl, f"pi5_{q}")
                pu = _iou_psum(2, sl, f"pu5_{q}")
                si = gp.tile([128, 1024], F16, tag="si5", name=f"si5_{q}")
                nc.scalar.activation(si[:], pi[:], SIG, bias=bi)
                tu = gp.tile([128, 1024], F16, tag="tu5", name=f"tu5_{q}")
                nc.scalar.activation(tu[:], pu[:], TANH, bias=bu)
                ct = gq.tile([128, 1024], F16, tag="ct5", name=f"ct5_{q}")
                nc.vector.tensor_mul(ct[:], si[:], tu[:])
                nc.vector.tensor_add(c5[:, sl], ct[:], fc5[:, sl])

            def l5top_p2(q):
                sl = slice(q * 1024, (q + 1) * 1024)
                po = _iou_psum(1, sl, f"po5_{q}")
                so = gp.tile([128, 1024], F16, tag="si5", name=f"so5_{q}")
                nc.scalar.activation(so[:], po[:], SIG, bias=bo)
                tct = gp.tile([128, 1024], F16, tag="tu5", name=f"tct5_{q}")
                nc.scalar.activation(tct[:], c5[:, sl], TANH)
                nc.vector.tensor_mul(h5[:, sl], so[:], tct[:])

            # ---- L4 forget gates + child sums (2 groups of 256 parents) ---
            def l4gates(g):
                cols = slice(g * 2048, (g + 1) * 2048)
                f4 = gp.tile([128, 2048], F16, tag="f4", name=f"f4l4_{g}")
                pf = psum.tile([128, 2048], F32, tag="pg", name=f"pf4_{g}")
                for s in range(0, 2048, 512):
                    nc.tensor.matmul(
                        pf[:, s : s + 512], W["wfh"][:],
                        h5[:, g * 2048 + s : g * 2048 + s + 512],
                        start=True, stop=False,
                    )
                for s in range(0, 2048, 512):
                    ps = g * 256 + s // 8
                    xb = (
                        x15[:, L4_OFF + ps : L4_OFF + ps + 64]
                        .rearrange("p (m o) -> p m o", o=1)
                        .broadcast_to([128, 64, 8])
                    )
                    nc.tensor.matmul(
                        pf[:, s : s + 512], W["wfx"][:], xb,
                        start=False, stop=True,
                    )
                nc.scalar.activation(f4[:], pf[:], SIG, bias=W["bf"][:])
                prod = gq.tile([128, 2048], F16, tag="prod", name=f"prod4_{g}")
                nc.vector.tensor_mul(prod[:], f4[:], c5[:, cols])
                psl = slice(g * 256, (g + 1) * 256)
                _tree(prod[:, 0:2048], 256, fc4[:, psl], f"fc4_{g}")
                _tree(h5[:, cols], 256, hs4[:, psl], f"hs4_{g}")

            # ---- L4 iou gates -> c4, sigmoid(o4) -> out -------------------
            def l4top():
                n = 512
                p = psum.tile([128, 3 * n], F32, tag="pg", name="p4")
                x_l = x15[:, L4_OFF : L4_OFF + n]
                for gate in range(3):
                    sl = slice(gate * n, (gate + 1) * n)
                    w = W["wioux"][:, gate * 128 : (gate + 1) * 128]
                    nc.tensor.matmul(p[:, sl], w, x_l, start=True, stop=False)
                    wh = W["wiouh"][:, gate * 128 : (gate + 1) * 128]
                    nc.tensor.matmul(p[:, sl], wh, hs4[:], start=False, stop=False)
                    br = W["biourow"][:, gate * 128 : (gate + 1) * 128]
                    nc.tensor.matmul(
                        p[:, sl], br, W["ones"][:, 0:n], start=False, stop=True
                    )
                sio = gq.tile([128, 2 * n], F16, tag="sio4", name="sio4")
                nc.scalar.activation(sio[:], p[:, 0 : 2 * n], SIG)
                tu = gq.tile([128, n], F16, tag="tu4f", name="tu4f")
                nc.scalar.activation(tu[:], p[:, 2 * n : 3 * n], TANH)
                ct = gq.tile([128, n], F16, tag="ct4", name="ct4")
                nc.vector.tensor_mul(ct[:], sio[:, 0:n], tu[:])
                nc.vector.tensor_add(outt[:, 0:512], ct[:], fc4[:])
                nc.vector.tensor_copy(outt[:, 512:1024], sio[:, n : 2 * n])
                nc.sync.dma_start(out_d, outt[:])

            # ---- schedule -------------------------------------------------
            for ch in range(N_CHUNKS):
                leaf_chunk(ch)
                if ch >= 1:
                    gates5(ch - 1)
                if ch == 3:
                    l5top_p1(0)
                elif ch == 4:
                    l5top_p2(0)
                    l5top_p1(1)
                elif ch == 5:
                    l5top_p2(1)
                elif ch == 6:
                    l5top_p1(2)
                    l4gates(0)
                elif ch == 7:
                    l5top_p2(2)

            gates5(7)
            l5top_p1(3)
            l5top_p2(3)
            l4gates(1)
            l4top()

    nc.compile()
    return nc


_NC_CACHE = None


def _get_nc():
    global _NC_CACHE
    if _NC_CACHE is None:
        _NC_CACHE = _build_subtree_kernel()
    return _NC_CACHE


def _sigmoid(x):
    return 1.0 / (1.0 + np.exp(-x))


def kernel(
    x, W_ioux, b_ioux, W_iouh, b_iouh, W_fx, b_fx, W_fh, b_fh, branch, depth
):
    global LAST_RESULTS
    assert int(branch) == BRANCH and int(depth) == DEPTH

    x = np.asarray(x, np.float32)
    W_ioux = np.asarray(W_ioux, np.float32)
    b_ioux = np.asarray(b_ioux, np.float32)
    W_iouh = np.asarray(W_iouh, np.float32)
    b_iouh = np.asarray(b_iouh, np.float32)
    W_fx = np.asarray(W_fx, np.float32)
    b_fx = np.asarray(b_fx, np.float32)
    W_fh = np.asarray(W_fh, np.float32)
    b_fh = np.asarray(b_fh, np.float32)

    wioux = np.ascontiguousarray(W_ioux.T.astype(np.float16))
    wiouh = np.ascontiguousarray(W_iouh.T.astype(np.float16))
    wfx = np.ascontiguousarray(W_fx.T.astype(np.float16))
    wfh = np.ascontiguousarray(W_fh.T.astype(np.float16))
    biou_full = b_ioux + b_iouh
    biou = np.ascontiguousarray(biou_full.reshape(3, 128).T.astype(np.float32))
    biourow = np.ascontiguousarray(biou_full.reshape(1, 384).astype(np.float16))
    bf = np.ascontiguousarray((b_fx + b_fh).reshape(128, 1).astype(np.float32))

    off = lambda l: (BRANCH**l - 1) // (BRANCH - 1)
    in_maps = []
    for c in range(BRANCH):
        parts = []
        for l in range(1, DEPTH + 1):
            sz = BRANCH ** (l - 1)
            parts.append(x[off(l) + c * sz : off(l) + (c + 1) * sz])
        xs_c = np.ascontiguousarray(
            np.concatenate(parts, axis=0).T.astype(np.float16)
        )
        in_maps.append(
            {
                "xs": xs_c,
                "wioux": wioux,
                "wiouh": wiouh,
                "wfx": wfx,
                "wfh": wfh,
                "biou": biou,
                "biourow": biourow,
                "bf": bf,
            }
        )

    nc = _get_nc()
    trace = os.environ.get("TREELSTM_TRACE") == "1"
    res = run_bass_kernel_spmd(nc, in_maps, core_ids=list(range(8)), trace=trace)
    LAST_RESULTS = res

    # device outputs: per core [128, 1024] fp32 = [c4 | sigmoid(o4)]
    c4 = np.stack([res.results[c]["out"][:, 0:512].T for c in range(8)])
    so4 = np.stack([res.results[c]["out"][:, 512:1024].T for c in range(8)])
    c_prev = c4.astype(np.float32)  # [8, 512, 128]
    h_prev = so4.astype(np.float32) * np.tanh(c_prev)

    # host-side sweep for levels 3..1 of each subtree (fp32)
    for lvl in (3, 2, 1):
        n_l = BRANCH ** (lvl - 1)  # per-core node count at this level
        x_l = np.stack(
            [
                x[off(lvl) + c * n_l : off(lvl) + (c + 1) * n_l]
                for c in range(BRANCH)
            ]
        )  # [8, n_l, 128]
        c_ch = c_prev.reshape(BRANCH, n_l, BRANCH, MEM)
        h_ch = h_prev.reshape(BRANCH, n_l, BRANCH, MEM)
        fx = x_l @ W_fx.T + b_fx
        f = _sigmoid(h_ch @ W_fh.T + b_fh + fx[:, :, None, :])
        fc_sum = (f * c_ch).sum(axis=2)
        h_sum = h_ch.sum(axis=2)
        iou = x_l @ W_ioux.T + b_ioux + h_sum @ W_iouh.T + b_iouh
        i, o, u = iou[..., 0:128], iou[..., 128:256], iou[..., 256:384]
        c_prev = _sigmoid(i) * np.tanh(u) + fc_sum
        h_prev = _sigmoid(o) * np.tanh(c_prev)

    # root node (children = the 8 subtree roots)
    c_ch = c_prev.reshape(BRANCH, MEM)
    h_ch = h_prev.reshape(BRANCH, MEM)
    x0 = x[0:1]
    h_sum = h_ch.sum(axis=0, keepdims=True)
    f = _sigmoid(h_ch @ W_fh.T + b_fh + (x0 @ W_fx.T + b_fx))
    fc_sum = (f * c_ch).sum(axis=0, keepdims=True)
    iou = x0 @ W_ioux.T + b_ioux + h_sum @ W_iouh.T + b_iouh
    i, o, u = iou[:, 0:128], iou[:, 128:256], iou[:, 256:384]
    c_root = _sigmoid(i) * np.tanh(u) + fc_sum
    h_root = _sigmoid(o) * np.tanh(c_root)
    return (c_root.astype(np.float32), h_root.astype(np.float32))
